# revision 18
# baseline (speedup 1.0000x reference)
"""Bass/Trainium2 kernel for DegreeOnlyFiltration (segment max + gather-divide).

Contract: kernel(**inputs) takes FULL inputs (node_deg [N] f32, sample_pos
[G+1] i32 CSR boundaries) and returns the FULL output node_deg / seg_max.

Strategy: segments are contiguous with uniform boundaries (sample_pos =
arange(G+1) * W); shard by whole segments across the 8 NeuronCores (pure data
parallel).  node_deg holds small integers, so the host losslessly recodes the
input to uint8 before staging and the device writes the quotient as float16
(rel err ~5e-4, well inside the 2e-2 gate); the host upcasts back to f32.
That cuts HBM traffic from 8 B/elem to 3 B/elem -- the kernel is pure
memory-bound, so this is the dominant win over an f32 in/out pipeline.

Per core: view the shard as [segs_per_core, W] u8, tile into [128, W/2]
chunks (one segment per partition row).  Chunk reduce_max (DVE/GPSIMD
alternating by tile), tensor_max combine, reciprocal, then per-partition-
scalar multiplies u8 -> f16 split across ACT and DVE, and chunk stores split
across the two HWDGE rings (SP for DVE-produced chunks so the in-order ACT
engine never waits on a DVE semaphore).
"""

import os

import numpy as np

import concourse.bacc as bacc
import concourse.mybir as mybir
import concourse.tile as tile
from concourse.bass_utils import run_bass_kernel_spmd

N_CORES = 8
P = 128  # SBUF partitions

# Populated after each traced run (test harness reads these).
LAST_EXEC_TIME_NS = None
LAST_RESULTS = None

_NC_CACHE = {}


def _build_u8_nc(segs_per_core: int, width: int, use_ttr: bool = True):
    """SPMD program: x [segs_per_core, width] u8 -> y = x / rowmax(x) as f16.

    One segment per partition row, n_tiles = segs_per_core / 128 tiles.
    Input DMAs all issue up front on the SP HWDGE ring.  Per tile: GPSIMD
    folds the two column halves with tensor_tensor max (free-axis
    tensor_reduce is DVE-only), DVE reduce_max + reciprocal on the folded
    half, then per-partition-scalar multiplies u8 -> f16 split across
    ACT/DVE, and chunk stores ride the ring of the engine that produced
    them (ACT ring / SP ring) so neither in-order engine stalls on the
    other's semaphore.
    """
    assert segs_per_core % P == 0
    assert width % 2 == 0
    n_tiles = segs_per_core // P
    cw = width // 2  # column chunk width
    f32 = mybir.dt.float32
    f16 = mybir.dt.float16
    u8 = mybir.dt.uint8

    nc = bacc.Bacc("TRN2", target_bir_lowering=False, debug=False,
                   num_devices=N_CORES, enable_partition_id=False,
                   enable_asserts=False)
    x = nc.dram_tensor("x", [segs_per_core, width], u8, kind="ExternalInput")
    y = nc.dram_tensor("y", [segs_per_core, width], f16, kind="ExternalOutput")

    with tile.TileContext(nc) as tc:
        with (
            tc.tile_pool(name="pin", bufs=1) as pin,
            tc.tile_pool(name="pscr", bufs=2) as pscr,
            tc.tile_pool(name="pout", bufs=1) as pout,
            tc.tile_pool(name="pm", bufs=2) as pm,
            tc.tile_pool(name="pr", bufs=n_tiles) as pr,
        ):
            # All input DMAs up front on the SP HWDGE ring (full tiles).
            tins = []
            for t in range(n_tiles):
                tin = pin.tile([P, width], u8, tag=f"tin{t}")
                nc.sync.dma_start(tin[:], x[t * P:(t + 1) * P, :])
                tins.append(tin)

            # DVE owns the max path: ONE tensor_tensor_reduce per tile
            # folds the two column halves (op0=max) and reduces the fold
            # on the fly (op1=max, accum_out) -- the whole segment max in
            # a single 2048-wide 1x pass, half the cost of reduce_max over
            # the full row.  The fold output goes to a discarded scratch.
            # The m pool recycles with bufs=2 and a SHARED tag, forcing
            # tile t's reciprocal before tile t+2's pass so ACT's muls
            # start as early as possible.
            rs = []
            for t in range(n_tiles):
                m = pm.tile([P, 1], f32, tag="m")
                if use_ttr:
                    scr = pscr.tile([P, cw], f16, tag="scr")
                    nc.vector.tensor_tensor_reduce(
                        out=scr[:], in0=tins[t][:, 0:cw],
                        in1=tins[t][:, cw:width], scale=1.0, scalar=0.0,
                        op0=mybir.AluOpType.max, op1=mybir.AluOpType.max,
                        accum_out=m[:])
                else:
                    nc.vector.reduce_max(m[:], tins[t][:],
                                         axis=mybir.AxisListType.X)
                r = pr.tile([P, 1], f32, tag=f"r.{t}")
                nc.vector.reciprocal(r[:], m[:])
                rs.append(r)

            # Muls: ACT (1 elem/cycle/lane) takes the first five chunks;
            # DVE (TensorScalar u8 measured 2x, ~0.63 ns/elem) takes the
            # last three after its reduce chain drains.  Stores all ride
            # the otherwise-idle SP ring: a ~0.6us DMA_DIRECT2D issue on
            # ACT would steal mul throughput.
            for t in range(n_tiles):
                s0 = t * P
                r = rs[t]
                to0 = pout.tile([P, cw], f16, tag=f"to0.{t}")
                to1 = pout.tile([P, cw], f16, tag=f"to1.{t}")
                dve0 = 2 * t >= 2 * n_tiles - 3      # chunk index >= 5
                dve1 = 2 * t + 1 >= 2 * n_tiles - 3
                if dve0:
                    nc.vector.tensor_scalar_mul(to0[:], tins[t][:, 0:cw], r[:])
                else:
                    nc.scalar.mul(to0[:], tins[t][:, 0:cw], r[:])
                if dve1:
                    nc.vector.tensor_scalar_mul(to1[:], tins[t][:, cw:width],
                                                r[:])
                else:
                    nc.scalar.mul(to1[:], tins[t][:, cw:width], r[:])
                nc.sync.dma_start(y[s0:s0 + P, 0:cw], to0[:])
                nc.sync.dma_start(y[s0:s0 + P, cw:width], to1[:])
    nc.compile()
    return nc


def _uniform_width(sample_pos: np.ndarray, n: int):
    """Return segment width W if boundaries are uniform (pos = arange*W)."""
    if sample_pos[0] != 0 or sample_pos[-1] != n:
        return None
    diffs = np.diff(sample_pos)
    if diffs.size == 0 or np.any(diffs != diffs[0]):
        return None
    return int(diffs[0])


def _host_fallback(node_deg: np.ndarray, sample_pos: np.ndarray) -> np.ndarray:
    """Exact mirror of the reference semantics for non-uniform boundaries."""
    import jax

    with jax.default_device(jax.devices("cpu")[0]):
        import jax.numpy as jnp

        deg = jnp.asarray(node_deg)
        pos = jnp.asarray(sample_pos)
        n = deg.shape[0]
        g = pos.shape[0] - 1
        seg_ids = jnp.searchsorted(pos[1:], jnp.arange(n, dtype=pos.dtype),
                                   side="right")
        seg_max = jax.ops.segment_max(deg, seg_ids, num_segments=g)
        return np.asarray(deg / seg_max[seg_ids])


def kernel(node_deg: np.ndarray, sample_pos: np.ndarray) -> np.ndarray:
    global LAST_EXEC_TIME_NS, LAST_RESULTS

    node_deg = np.asarray(node_deg, dtype=np.float32)
    sample_pos = np.asarray(sample_pos, dtype=np.int32)
    n = node_deg.shape[0]
    g = sample_pos.shape[0] - 1

    width = _uniform_width(sample_pos, n)
    if width is None or g % N_CORES != 0 or (g // N_CORES) % P != 0 \
            or width % 2 != 0 or width // 2 < 512:
        return _host_fallback(node_deg, sample_pos)

    # Lossless uint8 recode (degrees are small positive integers).
    deg_u8 = node_deg.astype(np.uint8)
    if not np.array_equal(deg_u8.astype(np.float32), node_deg):
        return _host_fallback(node_deg, sample_pos)

    segs_per_core = g // N_CORES

    shards = deg_u8.reshape(N_CORES, segs_per_core, width)
    in_maps = [{"x": shards[c]} for c in range(N_CORES)]
    trace = bool(int(os.environ.get("KERNEL_TRACE", "0")))

    res = None
    last_exc = None
    for use_ttr in (False,):  # TTR faults at execute on this runtime
        key = (segs_per_core, width, use_ttr)
        try:
            if key not in _NC_CACHE:
                _NC_CACHE[key] = _build_u8_nc(segs_per_core, width, use_ttr)
            nc = _NC_CACHE[key]
            try:
                res = run_bass_kernel_spmd(nc, in_maps,
                                           core_ids=list(range(N_CORES)),
                                           trace=trace)
            except Exception:
                if not trace:
                    raise
                # Trace post-processing can fail in sandboxes.
                res = run_bass_kernel_spmd(nc, in_maps,
                                           core_ids=list(range(N_CORES)),
                                           trace=False)
            break
        except Exception as e:  # noqa: BLE001 - fall back to reduce_max build
            last_exc = e
            continue
    if res is None:
        raise last_exc
    LAST_EXEC_TIME_NS = res.exec_time_ns
    LAST_RESULTS = res
    out = np.concatenate([res.results[c]["y"].reshape(-1)
                          for c in range(N_CORES)])
    return out.astype(np.float32, copy=False)


# revision 19
# speedup vs baseline: 1.0394x; 1.0394x over previous
"""Bass/Trainium2 kernel for DegreeOnlyFiltration (segment max + gather-divide).

Contract: kernel(**inputs) takes FULL inputs (node_deg [N] f32, sample_pos
[G+1] i32 CSR boundaries) and returns the FULL output node_deg / seg_max.

Strategy: segments are contiguous with uniform boundaries (sample_pos =
arange(G+1) * W); shard by whole segments across the 8 NeuronCores (pure data
parallel).  node_deg holds small integers, so the host losslessly recodes the
input to uint8 before staging and the device writes the quotient as float16
(rel err ~5e-4, well inside the 2e-2 gate); the host upcasts back to f32.
That cuts HBM traffic from 8 B/elem to 3 B/elem -- the kernel is pure
memory-bound, so this is the dominant win over an f32 in/out pipeline.

Per core: view the shard as [segs_per_core, W] u8, tile into [128, W/2]
chunks (one segment per partition row).  Chunk reduce_max (DVE/GPSIMD
alternating by tile), tensor_max combine, reciprocal, then per-partition-
scalar multiplies u8 -> f16 split across ACT and DVE, and chunk stores split
across the two HWDGE rings (SP for DVE-produced chunks so the in-order ACT
engine never waits on a DVE semaphore).
"""

import os

import numpy as np

import concourse.bacc as bacc
import concourse.mybir as mybir
import concourse.tile as tile
from concourse.bass_utils import run_bass_kernel_spmd

N_CORES = 8
P = 128  # SBUF partitions

# Populated after each traced run (test harness reads these).
LAST_EXEC_TIME_NS = None
LAST_RESULTS = None

_NC_CACHE = {}


def _build_u8_nc(segs_per_core: int, width: int, use_ttr: bool = True):
    """SPMD program: x [segs_per_core, width] u8 -> y = x / rowmax(x) as f16.

    One segment per partition row, n_tiles = segs_per_core / 128 tiles.
    Input DMAs all issue up front on the SP HWDGE ring.  Per tile: GPSIMD
    folds the two column halves with tensor_tensor max (free-axis
    tensor_reduce is DVE-only), DVE reduce_max + reciprocal on the folded
    half, then per-partition-scalar multiplies u8 -> f16 split across
    ACT/DVE, and chunk stores ride the ring of the engine that produced
    them (ACT ring / SP ring) so neither in-order engine stalls on the
    other's semaphore.
    """
    assert segs_per_core % P == 0
    assert width % 2 == 0
    n_tiles = segs_per_core // P
    cw = width // 2  # column chunk width
    f32 = mybir.dt.float32
    f16 = mybir.dt.float16
    u8 = mybir.dt.uint8

    nc = bacc.Bacc("TRN2", target_bir_lowering=False, debug=False,
                   num_devices=N_CORES, enable_partition_id=False,
                   enable_asserts=False)
    x = nc.dram_tensor("x", [segs_per_core, width], u8, kind="ExternalInput")
    y = nc.dram_tensor("y", [segs_per_core, width], f16, kind="ExternalOutput")

    with tile.TileContext(nc) as tc:
        with (
            tc.tile_pool(name="pin", bufs=1) as pin,
            tc.tile_pool(name="pscr", bufs=2) as pscr,
            tc.tile_pool(name="pout", bufs=1) as pout,
            tc.tile_pool(name="pm", bufs=1) as pm,
            tc.tile_pool(name="pr", bufs=n_tiles) as pr,
        ):
            # All input DMAs up front on the SP HWDGE ring (full tiles).
            tins = []
            for t in range(n_tiles):
                tin = pin.tile([P, width], u8, tag=f"tin{t}")
                nc.sync.dma_start(tin[:], x[t * P:(t + 1) * P, :])
                tins.append(tin)

            # DVE owns the max path: ONE tensor_tensor_reduce per tile
            # folds the two column halves (op0=max) and reduces the fold
            # on the fly (op1=max, accum_out) -- the whole segment max in
            # a single 2048-wide 1x pass, half the cost of reduce_max over
            # the full row.  The fold output goes to a discarded scratch.
            # The m pool recycles with bufs=2 and a SHARED tag, forcing
            # tile t's reciprocal before tile t+2's pass so ACT's muls
            # start as early as possible.
            rs = []
            for t in range(n_tiles):
                m = pm.tile([P, 1], f32, tag="m")
                if use_ttr:
                    scr = pscr.tile([P, cw], f16, tag="scr")
                    nc.vector.tensor_tensor_reduce(
                        out=scr[:], in0=tins[t][:, 0:cw],
                        in1=tins[t][:, cw:width], scale=1.0, scalar=0.0,
                        op0=mybir.AluOpType.max, op1=mybir.AluOpType.max,
                        accum_out=m[:])
                else:
                    nc.vector.reduce_max(m[:], tins[t][:],
                                         axis=mybir.AxisListType.X)
                r = pr.tile([P, 1], f32, tag=f"r.{t}")
                nc.vector.reciprocal(r[:], m[:])
                rs.append(r)

            # Muls: ACT (1 elem/cycle/lane) takes the first five chunks;
            # DVE (TensorScalar u8 measured 2x, ~0.63 ns/elem) takes the
            # last three after its reduce chain drains.  Stores all ride
            # the otherwise-idle SP ring: a ~0.6us DMA_DIRECT2D issue on
            # ACT would steal mul throughput.
            for t in range(n_tiles):
                s0 = t * P
                r = rs[t]
                to0 = pout.tile([P, cw], f16, tag=f"to0.{t}")
                to1 = pout.tile([P, cw], f16, tag=f"to1.{t}")
                dve0 = False                      # ACT: chunks 0..6
                dve1 = t == n_tiles - 1           # DVE: final chunk only
                if dve0:
                    nc.vector.tensor_scalar_mul(to0[:], tins[t][:, 0:cw], r[:])
                else:
                    nc.scalar.mul(to0[:], tins[t][:, 0:cw], r[:])
                if dve1:
                    nc.vector.tensor_scalar_mul(to1[:], tins[t][:, cw:width],
                                                r[:])
                else:
                    nc.scalar.mul(to1[:], tins[t][:, cw:width], r[:])
                nc.sync.dma_start(y[s0:s0 + P, 0:cw], to0[:])
                nc.sync.dma_start(y[s0:s0 + P, cw:width], to1[:])
    nc.compile()
    return nc


def _uniform_width(sample_pos: np.ndarray, n: int):
    """Return segment width W if boundaries are uniform (pos = arange*W)."""
    if sample_pos[0] != 0 or sample_pos[-1] != n:
        return None
    diffs = np.diff(sample_pos)
    if diffs.size == 0 or np.any(diffs != diffs[0]):
        return None
    return int(diffs[0])


def _host_fallback(node_deg: np.ndarray, sample_pos: np.ndarray) -> np.ndarray:
    """Exact mirror of the reference semantics for non-uniform boundaries."""
    import jax

    with jax.default_device(jax.devices("cpu")[0]):
        import jax.numpy as jnp

        deg = jnp.asarray(node_deg)
        pos = jnp.asarray(sample_pos)
        n = deg.shape[0]
        g = pos.shape[0] - 1
        seg_ids = jnp.searchsorted(pos[1:], jnp.arange(n, dtype=pos.dtype),
                                   side="right")
        seg_max = jax.ops.segment_max(deg, seg_ids, num_segments=g)
        return np.asarray(deg / seg_max[seg_ids])


def kernel(node_deg: np.ndarray, sample_pos: np.ndarray) -> np.ndarray:
    global LAST_EXEC_TIME_NS, LAST_RESULTS

    node_deg = np.asarray(node_deg, dtype=np.float32)
    sample_pos = np.asarray(sample_pos, dtype=np.int32)
    n = node_deg.shape[0]
    g = sample_pos.shape[0] - 1

    width = _uniform_width(sample_pos, n)
    if width is None or g % N_CORES != 0 or (g // N_CORES) % P != 0 \
            or width % 2 != 0 or width // 2 < 512:
        return _host_fallback(node_deg, sample_pos)

    # Lossless uint8 recode (degrees are small positive integers).
    deg_u8 = node_deg.astype(np.uint8)
    if not np.array_equal(deg_u8.astype(np.float32), node_deg):
        return _host_fallback(node_deg, sample_pos)

    segs_per_core = g // N_CORES

    shards = deg_u8.reshape(N_CORES, segs_per_core, width)
    in_maps = [{"x": shards[c]} for c in range(N_CORES)]
    trace = bool(int(os.environ.get("KERNEL_TRACE", "0")))

    res = None
    last_exc = None
    for use_ttr in (False,):  # TTR faults at execute on this runtime
        key = (segs_per_core, width, use_ttr)
        try:
            if key not in _NC_CACHE:
                _NC_CACHE[key] = _build_u8_nc(segs_per_core, width, use_ttr)
            nc = _NC_CACHE[key]
            try:
                res = run_bass_kernel_spmd(nc, in_maps,
                                           core_ids=list(range(N_CORES)),
                                           trace=trace)
            except Exception:
                if not trace:
                    raise
                # Trace post-processing can fail in sandboxes.
                res = run_bass_kernel_spmd(nc, in_maps,
                                           core_ids=list(range(N_CORES)),
                                           trace=False)
            break
        except Exception as e:  # noqa: BLE001 - fall back to reduce_max build
            last_exc = e
            continue
    if res is None:
        raise last_exc
    LAST_EXEC_TIME_NS = res.exec_time_ns
    LAST_RESULTS = res
    out = np.concatenate([res.results[c]["y"].reshape(-1)
                          for c in range(N_CORES)])
    return out.astype(np.float32, copy=False)
